# revision 20
# baseline (speedup 1.0000x reference)
"""Trainium2 Bass kernel: 16-head attention (B=2, S=2048, D=1024), 8-way sharded.

Sharding: core c handles batch b = c//4 and heads [4*(c%4), 4*(c%4)+4).
Megatron-style: Wq/Wk/Wv column-sharded (256 rows each), Wo row-sharded
(256 columns each); per-core partial outputs are summed on host.

Per-core device program (matmul inputs in fp16, fp32 PSUM accumulation):
  qT = (Wq_s @ x_q.T) + bq_s          [256, 2048]   (heads on partitions)
  kT = (Wk_s @ x_k.T) + bk_s          [256, 2048]
  v  = x_v @ Wv_s.T                   [2048, 256]   (no bias; folded on host)
  per head pair, per 512-col q-chunk, streaming over 16 key tiles:
    ST = k_h q_h.T (K=64 row-strip pair) -> exp(0.125*ST) on ACT ->
    ctxT_ext accumulate = [v_h | 1].T @ PT  (denominator fused as extra row:
    even head ones at col 64, odd head ones at col 0, dims at 64:128)
  normalize: stage psum to SBUF, reciprocal_approx_fast on the denominator
  rows, K=1 matmul broadcast, scalar_tensor_tensor multiply -> ctxT (fp16)
  out_partial = ctxT.T @ Wo_s.T       [2048, 1024]
Host: out[b] = sum of 4 partials + bo + Wo @ bv.
"""

import sys

sys.path.insert(0, "/opt/trn_rl_repo")

import functools

import numpy as np

import concourse.bass as bass
import concourse.mybir as mybir
import concourse.tile as tile
from concourse.bass_utils import run_bass_kernel_spmd
from concourse.vector_clock import ScopedClock, VectorClock

P = 128
S = 2048
D = 1024
M = 256  # local head dims per core (4 heads x 64)
NSQ = 4  # 512-wide query chunks
NSK = 16  # 128-row key tiles
NDC = 8  # 128-row chunks of the model dim
F32 = mybir.dt.float32
F16 = mybir.dt.float16
EXP = mybir.ActivationFunctionType.Exp
MUL = mybir.AluOpType.mult


def _drain_and_barrier_split(self, tick_clock, wait_clock):
    # The stock tail emits one Drain carrying a sem wait per live processor;
    # this walrus build rejects >1 sync wait on an instruction. Emit one
    # Drain per processor instead, each carrying a single wait.
    nc = self.nc
    vclock = tick_clock.global_clock
    n = len(vclock)
    for i in range(n):
        t = vclock[i]
        if t > 0:
            vc = VectorClock([0] * n)
            vc.require_at_least(i, t)
            inst = nc.sync.drain()
            wait_clock.add_sem_waits(inst.ins, ScopedClock({None: vc}))
    nc.all_engine_barrier()
    assert self.sems is not None
    popped = nc._tile_sem_poison_stack.pop()
    assert popped is self._sem_poison
    nc.clear_and_free_semaphores(list(self.sems.allocated().values()))
    nc.all_engine_barrier()


tile.TileContext._drain_and_barrier = _drain_and_barrier_split


def _split_multi_waits(nc, cap=1):
    """This walrus build rejects instructions carrying more than one sync
    wait. Move surplus waits onto nop instructions inserted just before the
    offending instruction on the same engine (engine FIFO preserves order)."""
    import bass_rust

    n_nops = 0
    for f in nc.m.functions:
        for blk in f.blocks:
            insts = blk.instructions  # live list view
            i = 0
            while i < len(insts):
                inst = insts[i]
                si = inst.sync_info
                if si is not None and len(si.on_wait) > cap:
                    waits = list(si.on_wait)
                    extra, keep = waits[:-cap], waits[-cap:]
                    pos = i
                    for j in range(0, len(extra), cap):
                        chunk = extra[j : j + cap]
                        bi = nc.engines[inst.engine].nop(nofuse=True)
                        nop_inst = bi.ins
                        tail = nc.cur_bb.bb.instructions
                        assert tail[-1].name == nop_inst.name
                        tail.pop()
                        nop_inst.sync_info = bass_rust.SyncInfo(
                            on_wait=chunk, on_update=[]
                        )
                        insts.insert(pos, nop_inst)
                        pos += 1
                        i += 1
                        n_nops += 1
                    inst.sync_info = bass_rust.SyncInfo(
                        on_wait=keep, on_update=list(si.on_update)
                    )
                i += 1
    return n_nops


@functools.lru_cache(maxsize=1)
def _build():
    nc = bass.Bass()
    xq = nc.declare_dram_parameter("xq", [D, S], F16, isOutput=False)
    xk = nc.declare_dram_parameter("xk", [D, S], F16, isOutput=False)
    xv = nc.declare_dram_parameter("xv", [D, S], F16, isOutput=False)
    wq = nc.declare_dram_parameter("wq", [D, M], F16, isOutput=False)
    wk = nc.declare_dram_parameter("wk", [D, M], F16, isOutput=False)
    wv = nc.declare_dram_parameter("wv", [D, M], F16, isOutput=False)
    wo = nc.declare_dram_parameter("wo", [M, D], F16, isOutput=False)
    bq = nc.declare_dram_parameter("bq", [M], F32, isOutput=False)
    bk = nc.declare_dram_parameter("bk", [M], F32, isOutput=False)
    out = nc.declare_dram_parameter("out", [S, D], F32, isOutput=True)

    with tile.TileContext(nc) as tc:
        with tc.tile_pool(name="res", bufs=1) as res:
            wq_sb = res.tile([P, NDC, M], F16, name="wq_sb")
            wk_sb = res.tile([P, NDC, M], F16, name="wk_sb")
            wv_sb = res.tile([P, NDC, M], F16, name="wv_sb")
            wo_sb = res.tile([P, 2, D], F16, name="wo_sb")
            bq_sb = res.tile([P, 2], F32, name="bq_sb")
            bk_sb = res.tile([P, 2], F32, name="bk_sb")
            ones_sb = res.tile([P, P], F16, name="ones_sb")
            qT_sb = res.tile([P, 2, S], F16, name="qT_sb")
            kT_sb = res.tile([P, 2, S], F16, name="kT_sb")
            ctxT_sb = res.tile([P, 2, S], F16, name="ctxT_sb")
            # per key-tile, per pair: [v_even | ones] (65 cols) and a
            # 128-wide odd slab: col 0 = ones, cols 64:128 = v_odd dims.
            v_e = res.tile([P, NSK, 2, P], F16, name="v_e")
            v_o = res.tile([P, NSK, 2, P], F16, name="v_o")

            nc.sync.dma_start(out=wq_sb[:], in_=wq.rearrange("(c p) m -> p c m", p=P))
            nc.sync.dma_start(out=wk_sb[:], in_=wk.rearrange("(c p) m -> p c m", p=P))
            nc.sync.dma_start(out=wv_sb[:], in_=wv.rearrange("(c p) m -> p c m", p=P))
            nc.sync.dma_start(out=wo_sb[:], in_=wo.rearrange("(t p) j -> p t j", p=P))
            nc.sync.dma_start(out=bq_sb[:], in_=bq.rearrange("(t p) -> p t", p=P))
            nc.sync.dma_start(out=bk_sb[:], in_=bk.rearrange("(t p) -> p t", p=P))
            ones_pat = np.ones((P, P), np.float16)
            vo_pat = np.zeros((P, NSK, 2, 64), np.float16)
            vo_pat[:, :, :, 0] = 1.0
            ve_pad = np.zeros((P, NSK, 2, 64), np.float16)
            ve_pad[:, :, :, 0] = 1.0
            c_ones = nc.inline_tensor(ones_pat, name="c_ones")
            c_vo = nc.inline_tensor(vo_pat, name="c_vo")
            c_ve = nc.inline_tensor(ve_pad, name="c_ve")
            nc.sync.dma_start(out=ones_sb[:], in_=c_ones[:])
            nc.sync.dma_start(out=v_o[:, :, :, 0:64], in_=c_vo[:])
            nc.sync.dma_start(out=v_e[:, :, :, 64:P], in_=c_ve[:])

            # ---- projections ----
            with tc.tile_pool(name="xs", bufs=12) as xs:
              with tc.tile_pool(name="pj", bufs=4, space="PSUM") as pj:
                for xin, w_sb, b_sb, dst in (
                    (xq, wq_sb, bq_sb, qT_sb),
                    (xk, wk_sb, bk_sb, kT_sb),
                ):
                    for c in range(NSQ):
                        xtiles = []
                        for dc in range(NDC):
                            xt = xs.tile([P, 512], F16, name="xslice")
                            nc.sync.dma_start(
                                out=xt[:],
                                in_=xin[
                                    dc * P : (dc + 1) * P, c * 512 : (c + 1) * 512
                                ],
                            )
                            xtiles.append(xt)
                        pss2 = [
                            pj.tile([P, 512], F32, name="pjqk", tag="pjqk")
                            for _ in range(2)
                        ]
                        for dc in range(NDC):
                            for t in range(2):
                                nc.tensor.matmul(
                                    pss2[t][:],
                                    lhsT=w_sb[:, dc, t * P : (t + 1) * P],
                                    rhs=xtiles[dc][:],
                                    start=(dc == 0),
                                    stop=(dc == NDC - 1),
                                )
                        for t in range(2):
                            nc.vector.tensor_scalar_add(
                                dst[:, t, c * 512 : (c + 1) * 512],
                                pss2[t][:],
                                b_sb[:, t : t + 1],
                            )
              with tc.tile_pool(name="pjv", bufs=8, space="PSUM") as pjv:
                for half in range(2):
                    psvs = [
                        pjv.tile([P, M], F32, name="pjv", tag="pjv")
                        for _ in range(8)
                    ]
                    for dc in range(NDC):
                        xv_t = xs.tile([P, 1024], F16, name="xvslice")
                        nc.sync.dma_start(
                            out=xv_t[:],
                            in_=xv[
                                dc * P : (dc + 1) * P,
                                half * 1024 : (half + 1) * 1024,
                            ],
                        )
                        for j in range(8):
                            nc.tensor.matmul(
                                psvs[j][:],
                                lhsT=xv_t[:, j * P : (j + 1) * P],
                                rhs=wv_sb[:, dc, :],
                                start=(dc == 0),
                                stop=(dc == NDC - 1),
                            )
                    for j in range(8):
                        st = half * 8 + j
                        psv_r = psvs[j].rearrange("p (t m) -> p t m", t=2)
                        nc.vector.tensor_copy(v_e[:, st, :, 0:64], psv_r[:, :, 0:64])
                        nc.vector.tensor_copy(v_o[:, st, :, 64:P], psv_r[:, :, 64:P])

            # ---- attention ----
            with (
                tc.tile_pool(name="ptp", bufs=4) as ptp,
                tc.tile_pool(name="rcpp", bufs=3) as rcpp,
                tc.tile_pool(name="ps_s", bufs=2, space="PSUM") as ps_s,
                tc.tile_pool(name="ps_c", bufs=2, space="PSUM") as ps_c,
                tc.tile_pool(name="ps_b", bufs=2, space="PSUM") as ps_b,
                tc.tile_pool(name="osb", bufs=4) as osb,
            ):
                def make_flush(t, cs, u):
                    # Deferred tail of the previous (t, c) iteration: emitted
                    # a few sk-steps into the NEXT iteration so the PE never
                    # stalls waiting for the reciprocal chain on DVE.
                    def flush():
                        pb_e = ps_b.tile([P, 512], F32, name="bc", tag="bc")
                        pb_o = ps_b.tile([P, 512], F32, name="bc", tag="bc")
                        nc.tensor.matmul(
                            pb_e[0:64, :],
                            lhsT=ones_sb[64:65, 0:64],
                            rhs=rc[64:65, 0:512],
                            start=True,
                            stop=True,
                        )
                        nc.tensor.matmul(
                            pb_o[:, :],
                            lhsT=ones_sb[0:1, :],
                            rhs=rc[0:1, 512:1024],
                            start=True,
                            stop=True,
                        )
                        nc.vector.scalar_tensor_tensor(
                            out=ctxT_sb[0:64, t, cs],
                            in0=pb_e[0:64, :],
                            scalar=1.0,
                            in1=u[0:64, 0:512],
                            op0=MUL,
                            op1=MUL,
                        )
                        nc.vector.scalar_tensor_tensor(
                            out=ctxT_sb[64:P, t, cs],
                            in0=pb_o[64:P, :],
                            scalar=1.0,
                            in1=u[64:P, 512:1024],
                            op0=MUL,
                            op1=MUL,
                        )

                    rc32 = rcpp.tile([P, 1024], F32, name="rc32", tag="rc32")
                    nc.vector.reciprocal(rc32[64:65, 0:512], u[64:65, 0:512])
                    nc.vector.reciprocal(rc32[0:1, 512:1024], u[0:1, 512:1024])
                    rc = rcpp.tile([P, 1024], F16, name="rc", tag="rc")
                    nc.vector.tensor_copy(rc[64:65, 0:512], rc32[64:65, 0:512])
                    nc.vector.tensor_copy(rc[0:1, 512:1024], rc32[0:1, 512:1024])
                    return flush

                def make_outproj(c):
                    # 8 psum-group emitters for the s-range of chunk c;
                    # consumed one per sk-step inside a later iteration so
                    # PE work fills ACT-bound bubbles.
                    items = []
                    for st in range(4 * c, 4 * c + 4):
                        for jc in range(2):

                            def emit(st=st, jc=jc):
                                po = ps_b.tile([P, 512], F32, name="bc", tag="bc")
                                for tt in range(2):
                                    nc.tensor.matmul(
                                        po[:],
                                        lhsT=ctxT_sb[:, tt, st * P : (st + 1) * P],
                                        rhs=wo_sb[:, tt, jc * 512 : (jc + 1) * 512],
                                        start=(tt == 0),
                                        stop=(tt == 1),
                                    )
                                ot = osb.tile([P, 512], F32, name="ot")
                                nc.vector.tensor_copy(ot[:], po[:])
                                nc.sync.dma_start(
                                    out=out[
                                        st * P : (st + 1) * P,
                                        jc * 512 : (jc + 1) * 512,
                                    ],
                                    in_=ot[:],
                                )

                            items.append(emit)
                    return items

                deferred = []  # [ready_iter, fn] FIFO, fired >=6 sk-steps in
                idx = 0
                for c in range(NSQ):
                    for t in range(2):
                        cs = slice(c * 512, (c + 1) * 512)
                        pc_e = ps_c.tile([P, 512], F32, name="ctx", tag="ctx")
                        pc_o = ps_c.tile([P, 512], F32, name="ctx", tag="ctx")
                        for sk in range(NSK):
                            ks = slice(sk * P, (sk + 1) * P)
                            pss = ps_s.tile([P, 1024], F32, name="scores")
                            nc.tensor.matmul(
                                pss[:, 0:512],
                                lhsT=kT_sb[0:64, t, ks],
                                rhs=qT_sb[0:64, t, cs],
                                start=True,
                                stop=True,
                            )
                            nc.tensor.matmul(
                                pss[:, 512:1024],
                                lhsT=kT_sb[64:P, t, ks],
                                rhs=qT_sb[64:P, t, cs],
                                start=True,
                                stop=True,
                            )
                            pt = ptp.tile([P, 1024], F16, name="pt")
                            nc.scalar.activation(pt[:], pss[:], EXP, scale=0.125)
                            nc.tensor.matmul(
                                pc_e[:, :],
                                lhsT=v_e[:, sk, t, :],
                                rhs=pt[:, 0:512],
                                start=(sk == 0),
                                stop=(sk == NSK - 1),
                            )
                            nc.tensor.matmul(
                                pc_o[:, :],
                                lhsT=v_o[:, sk, t, :],
                                rhs=pt[:, 512:1024],
                                start=(sk == 0),
                                stop=(sk == NSK - 1),
                            )
                            if sk >= 6:
                                for d in deferred:
                                    if d[0] <= idx:
                                        d[1]()
                                        deferred.remove(d)
                                        break
                        # stage psum to SBUF (frees the ctx banks fast),
                        # start the reciprocals, defer the rest two iters
                        u = rcpp.tile([P, 1024], F32, name="u", tag="u")
                        nc.vector.tensor_copy(u[0:65, 0:512], pc_e[0:65, :])
                        nc.vector.tensor_copy(u[:, 512:1024], pc_o[:, :])
                        deferred.append([idx + 2, make_flush(t, cs, u)])
                        if t == 1:
                            for fn in make_outproj(c):
                                deferred.append([idx + 2, fn])
                        idx += 1
                for d in deferred:
                    d[1]()

    n = _split_multi_waits(nc)
    print(f"[kernel] split {n} multi-wait instructions into nops", flush=True)
    return nc


def _in_maps(query, key, value, Wq, bq, Wk, bk, Wv, Wo):
    maps = []
    for core in range(8):
        b = core // 4
        r0 = (core % 4) * M
        r1 = r0 + M
        maps.append(
            {
                "xq": np.ascontiguousarray(query[b].T).astype(np.float16),
                "xk": np.ascontiguousarray(key[b].T).astype(np.float16),
                "xv": np.ascontiguousarray(value[b].T).astype(np.float16),
                "wq": np.ascontiguousarray(Wq[r0:r1].T).astype(np.float16),
                "wk": np.ascontiguousarray(Wk[r0:r1].T).astype(np.float16),
                "wv": np.ascontiguousarray(Wv[r0:r1].T).astype(np.float16),
                "wo": np.ascontiguousarray(Wo[:, r0:r1].T).astype(np.float16),
                "bq": np.ascontiguousarray(bq[r0:r1]),
                "bk": np.ascontiguousarray(bk[r0:r1]),
            }
        )
    return maps


def _gather(results, Wo, bv, bo):
    corr = (bo + Wo @ bv).astype(np.float32)
    full = np.empty((2, S, D), np.float32)
    for b in range(2):
        acc = results[4 * b]["out"].astype(np.float32).copy()
        for i in range(1, 4):
            acc += results[4 * b + i]["out"]
        full[b] = acc + corr[None, :]
    return full


def kernel(query, key, value, Wq, bq, Wk, bk, Wv, bv, Wo, bo, _run_kwargs=None):
    query, key, value, Wq, bq, Wk, bk, Wv, bv, Wo, bo = (
        np.asarray(a, np.float32)
        for a in (query, key, value, Wq, bq, Wk, bk, Wv, bv, Wo, bo)
    )
    nc = _build()
    maps = _in_maps(query, key, value, Wq, bq, Wk, bk, Wv, Wo)
    res = run_bass_kernel_spmd(nc, maps, core_ids=list(range(8)), **(_run_kwargs or {}))
    out = _gather(res.results, Wo, bv, bo)
    if _run_kwargs:
        kernel.last_results = res
    return out


# revision 21
# speedup vs baseline: 1.0264x; 1.0264x over previous
"""Trainium2 Bass kernel: 16-head attention (B=2, S=2048, D=1024), 8-way sharded.

Sharding: core c handles batch b = c//4 and heads [4*(c%4), 4*(c%4)+4).
Megatron-style: Wq/Wk/Wv column-sharded (256 rows each), Wo row-sharded
(256 columns each); per-core partial outputs are summed on host.

Per-core device program (matmul inputs in fp16, fp32 PSUM accumulation):
  qT = (Wq_s @ x_q.T) + bq_s          [256, 2048]   (heads on partitions)
  kT = (Wk_s @ x_k.T) + bk_s          [256, 2048]
  v  = x_v @ Wv_s.T                   [2048, 256]   (no bias; folded on host)
  per head pair, per 512-col q-chunk, streaming over 16 key tiles:
    ST = k_h q_h.T (K=64 row-strip pair) -> exp(0.125*ST) on ACT ->
    ctxT_ext accumulate = [v_h | 1].T @ PT  (denominator fused as extra row:
    even head ones at col 64, odd head ones at col 0, dims at 64:128)
  normalize: stage psum to SBUF, reciprocal_approx_fast on the denominator
  rows, K=1 matmul broadcast, scalar_tensor_tensor multiply -> ctxT (fp16)
  out_partial = ctxT.T @ Wo_s.T       [2048, 1024]
Host: out[b] = sum of 4 partials + bo + Wo @ bv.
"""

import sys

sys.path.insert(0, "/opt/trn_rl_repo")

import functools

import numpy as np

import concourse.bass as bass
import concourse.mybir as mybir
import concourse.tile as tile
from concourse.bass_utils import run_bass_kernel_spmd
from concourse.vector_clock import ScopedClock, VectorClock

P = 128
S = 2048
D = 1024
M = 256  # local head dims per core (4 heads x 64)
NSQ = 4  # 512-wide query chunks
NSK = 16  # 128-row key tiles
NDC = 8  # 128-row chunks of the model dim
F32 = mybir.dt.float32
F16 = mybir.dt.float16
EXP = mybir.ActivationFunctionType.Exp
MUL = mybir.AluOpType.mult


def _drain_and_barrier_split(self, tick_clock, wait_clock):
    # The stock tail emits one Drain carrying a sem wait per live processor;
    # this walrus build rejects >1 sync wait on an instruction. Emit one
    # Drain per processor instead, each carrying a single wait.
    nc = self.nc
    vclock = tick_clock.global_clock
    n = len(vclock)
    for i in range(n):
        t = vclock[i]
        if t > 0:
            vc = VectorClock([0] * n)
            vc.require_at_least(i, t)
            inst = nc.sync.drain()
            wait_clock.add_sem_waits(inst.ins, ScopedClock({None: vc}))
    nc.all_engine_barrier()
    assert self.sems is not None
    popped = nc._tile_sem_poison_stack.pop()
    assert popped is self._sem_poison
    nc.clear_and_free_semaphores(list(self.sems.allocated().values()))
    nc.all_engine_barrier()


tile.TileContext._drain_and_barrier = _drain_and_barrier_split


def _split_multi_waits(nc, cap=1):
    """This walrus build rejects instructions carrying more than one sync
    wait. Move surplus waits onto nop instructions inserted just before the
    offending instruction on the same engine (engine FIFO preserves order)."""
    import bass_rust

    n_nops = 0
    for f in nc.m.functions:
        for blk in f.blocks:
            insts = blk.instructions  # live list view
            i = 0
            while i < len(insts):
                inst = insts[i]
                si = inst.sync_info
                if si is not None and len(si.on_wait) > cap:
                    waits = list(si.on_wait)
                    extra, keep = waits[:-cap], waits[-cap:]
                    pos = i
                    for j in range(0, len(extra), cap):
                        chunk = extra[j : j + cap]
                        bi = nc.engines[inst.engine].nop(nofuse=True)
                        nop_inst = bi.ins
                        tail = nc.cur_bb.bb.instructions
                        assert tail[-1].name == nop_inst.name
                        tail.pop()
                        nop_inst.sync_info = bass_rust.SyncInfo(
                            on_wait=chunk, on_update=[]
                        )
                        insts.insert(pos, nop_inst)
                        pos += 1
                        i += 1
                        n_nops += 1
                    inst.sync_info = bass_rust.SyncInfo(
                        on_wait=keep, on_update=list(si.on_update)
                    )
                i += 1
    return n_nops


@functools.lru_cache(maxsize=1)
def _build():
    nc = bass.Bass()
    xq = nc.declare_dram_parameter("xq", [D, S], F16, isOutput=False)
    xk = nc.declare_dram_parameter("xk", [D, S], F16, isOutput=False)
    xv = nc.declare_dram_parameter("xv", [D, S], F16, isOutput=False)
    wq = nc.declare_dram_parameter("wq", [D, M], F16, isOutput=False)
    wk = nc.declare_dram_parameter("wk", [D, M], F16, isOutput=False)
    wv = nc.declare_dram_parameter("wv", [D, M], F16, isOutput=False)
    wo = nc.declare_dram_parameter("wo", [M, D], F16, isOutput=False)
    bq = nc.declare_dram_parameter("bq", [M], F32, isOutput=False)
    bk = nc.declare_dram_parameter("bk", [M], F32, isOutput=False)
    out = nc.declare_dram_parameter("out", [S, D], F32, isOutput=True)

    with tile.TileContext(nc) as tc:
        with tc.tile_pool(name="res", bufs=1) as res:
            wq_sb = res.tile([P, NDC, M], F16, name="wq_sb")
            wk_sb = res.tile([P, NDC, M], F16, name="wk_sb")
            wv_sb = res.tile([P, NDC, M], F16, name="wv_sb")
            wo_sb = res.tile([P, 2, D], F16, name="wo_sb")
            bq_sb = res.tile([P, 2], F32, name="bq_sb")
            bk_sb = res.tile([P, 2], F32, name="bk_sb")
            ones_sb = res.tile([P, P], F16, name="ones_sb")
            qT_sb = res.tile([P, 2, S], F16, name="qT_sb")
            kT_sb = res.tile([P, 2, S], F16, name="kT_sb")
            ctxT_sb = res.tile([P, 2, S], F16, name="ctxT_sb")
            # per key-tile, per pair: [v_even | ones] (65 cols) and a
            # 128-wide odd slab: col 0 = ones, cols 64:128 = v_odd dims.
            v_e = res.tile([P, NSK, 2, P], F16, name="v_e")
            v_o = res.tile([P, NSK, 2, P], F16, name="v_o")

            nc.sync.dma_start(out=wq_sb[:], in_=wq.rearrange("(c p) m -> p c m", p=P))
            nc.sync.dma_start(out=wk_sb[:], in_=wk.rearrange("(c p) m -> p c m", p=P))
            nc.sync.dma_start(out=wv_sb[:], in_=wv.rearrange("(c p) m -> p c m", p=P))
            nc.sync.dma_start(out=wo_sb[:], in_=wo.rearrange("(t p) j -> p t j", p=P))
            nc.sync.dma_start(out=bq_sb[:], in_=bq.rearrange("(t p) -> p t", p=P))
            nc.sync.dma_start(out=bk_sb[:], in_=bk.rearrange("(t p) -> p t", p=P))
            ones_pat = np.ones((P, P), np.float16)
            vo_pat = np.zeros((P, NSK, 2, 64), np.float16)
            vo_pat[:, :, :, 0] = 1.0
            ve_pad = np.zeros((P, NSK, 2, 64), np.float16)
            ve_pad[:, :, :, 0] = 1.0
            c_ones = nc.inline_tensor(ones_pat, name="c_ones")
            c_vo = nc.inline_tensor(vo_pat, name="c_vo")
            c_ve = nc.inline_tensor(ve_pad, name="c_ve")
            nc.sync.dma_start(out=ones_sb[:], in_=c_ones[:])
            nc.sync.dma_start(out=v_o[:, :, :, 0:64], in_=c_vo[:])
            nc.sync.dma_start(out=v_e[:, :, :, 64:P], in_=c_ve[:])

            # ---- projections ----
            with tc.tile_pool(name="xs", bufs=12) as xs:
              with tc.tile_pool(name="pj", bufs=4, space="PSUM") as pj:
                for xin, w_sb, b_sb, dst in (
                    (xq, wq_sb, bq_sb, qT_sb),
                    (xk, wk_sb, bk_sb, kT_sb),
                ):
                    for c in range(NSQ):
                        xtiles = []
                        for dc in range(NDC):
                            xt = xs.tile([P, 512], F16, name="xslice")
                            nc.sync.dma_start(
                                out=xt[:],
                                in_=xin[
                                    dc * P : (dc + 1) * P, c * 512 : (c + 1) * 512
                                ],
                            )
                            xtiles.append(xt)
                        pss2 = [
                            pj.tile([P, 512], F32, name="pjqk", tag="pjqk")
                            for _ in range(2)
                        ]
                        for dc in range(NDC):
                            for t in range(2):
                                nc.tensor.matmul(
                                    pss2[t][:],
                                    lhsT=w_sb[:, dc, t * P : (t + 1) * P],
                                    rhs=xtiles[dc][:],
                                    start=(dc == 0),
                                    stop=(dc == NDC - 1),
                                )
                        for t in range(2):
                            nc.vector.tensor_scalar_add(
                                dst[:, t, c * 512 : (c + 1) * 512],
                                pss2[t][:],
                                b_sb[:, t : t + 1],
                            )
              with tc.tile_pool(name="pjv", bufs=8, space="PSUM") as pjv:
                for half in range(2):
                    psvs = [
                        pjv.tile([P, M], F32, name="pjv", tag="pjv")
                        for _ in range(8)
                    ]
                    for dc in range(NDC):
                        xv_t = xs.tile([P, 1024], F16, name="xvslice")
                        nc.sync.dma_start(
                            out=xv_t[:],
                            in_=xv[
                                dc * P : (dc + 1) * P,
                                half * 1024 : (half + 1) * 1024,
                            ],
                        )
                        for j in range(8):
                            nc.tensor.matmul(
                                psvs[j][:],
                                lhsT=xv_t[:, j * P : (j + 1) * P],
                                rhs=wv_sb[:, dc, :],
                                start=(dc == 0),
                                stop=(dc == NDC - 1),
                            )
                    for j in range(8):
                        st = half * 8 + j
                        psv_r = psvs[j].rearrange("p (t m) -> p t m", t=2)
                        nc.vector.tensor_copy(v_e[:, st, :, 0:64], psv_r[:, :, 0:64])
                        nc.vector.tensor_copy(v_o[:, st, :, 64:P], psv_r[:, :, 64:P])

            # ---- attention ----
            with (
                tc.tile_pool(name="ptp", bufs=4) as ptp,
                tc.tile_pool(name="rcpp", bufs=3) as rcpp,
                tc.tile_pool(name="ps_s", bufs=2, space="PSUM") as ps_s,
                tc.tile_pool(name="ps_c", bufs=2, space="PSUM") as ps_c,
                tc.tile_pool(name="ps_b", bufs=2, space="PSUM") as ps_b,
                tc.tile_pool(name="osb", bufs=4) as osb,
            ):
                def make_flush(t, cs, u):
                    # Deferred tail of the previous (t, c) iteration: emitted
                    # a few sk-steps into the NEXT iteration so the PE never
                    # stalls waiting for the reciprocal chain on DVE.
                    def flush():
                        pb_e = ps_b.tile([P, 512], F32, name="bc", tag="bc")
                        pb_o = ps_b.tile([P, 512], F32, name="bc", tag="bc")
                        nc.tensor.matmul(
                            pb_e[0:64, :],
                            lhsT=ones_sb[64:65, 0:64],
                            rhs=rc[64:65, 0:512],
                            start=True,
                            stop=True,
                        )
                        nc.tensor.matmul(
                            pb_o[:, :],
                            lhsT=ones_sb[0:1, :],
                            rhs=rc[0:1, 512:1024],
                            start=True,
                            stop=True,
                        )
                        nc.vector.scalar_tensor_tensor(
                            out=ctxT_sb[0:64, t, cs],
                            in0=pb_e[0:64, :],
                            scalar=1.0,
                            in1=u[0:64, 0:512],
                            op0=MUL,
                            op1=MUL,
                        )
                        nc.vector.scalar_tensor_tensor(
                            out=ctxT_sb[64:P, t, cs],
                            in0=pb_o[64:P, :],
                            scalar=1.0,
                            in1=u[64:P, 512:1024],
                            op0=MUL,
                            op1=MUL,
                        )

                    rc32 = rcpp.tile([P, 1024], F32, name="rc32", tag="rc32")
                    nc.vector.reciprocal(rc32[64:65, 0:512], u[64:65, 0:512])
                    nc.vector.reciprocal(rc32[0:1, 512:1024], u[0:1, 512:1024])
                    rc = rcpp.tile([P, 1024], F16, name="rc", tag="rc")
                    nc.vector.tensor_copy(rc[64:65, 0:512], rc32[64:65, 0:512])
                    nc.vector.tensor_copy(rc[0:1, 512:1024], rc32[0:1, 512:1024])
                    return flush

                def make_outproj(c):
                    # 8 psum-group emitters for the s-range of chunk c;
                    # consumed one per sk-step inside a later iteration so
                    # PE work fills ACT-bound bubbles.
                    items = []
                    for st in range(4 * c, 4 * c + 4):
                        for jc in range(2):

                            def emit(st=st, jc=jc):
                                po = ps_b.tile([P, 512], F32, name="bc", tag="bc")
                                for tt in range(2):
                                    nc.tensor.matmul(
                                        po[:],
                                        lhsT=ctxT_sb[:, tt, st * P : (st + 1) * P],
                                        rhs=wo_sb[:, tt, jc * 512 : (jc + 1) * 512],
                                        start=(tt == 0),
                                        stop=(tt == 1),
                                    )
                                ot = osb.tile([P, 512], F32, name="ot")
                                nc.scalar.copy(ot[:], po[:])
                                nc.sync.dma_start(
                                    out=out[
                                        st * P : (st + 1) * P,
                                        jc * 512 : (jc + 1) * 512,
                                    ],
                                    in_=ot[:],
                                )

                            items.append(emit)
                    return items

                deferred = []  # [ready_iter, fn] FIFO, fired >=6 sk-steps in
                idx = 0
                for c in range(NSQ):
                    for t in range(2):
                        cs = slice(c * 512, (c + 1) * 512)
                        pc_e = ps_c.tile([P, 512], F32, name="ctx", tag="ctx")
                        pc_o = ps_c.tile([P, 512], F32, name="ctx", tag="ctx")
                        for sk in range(NSK):
                            ks = slice(sk * P, (sk + 1) * P)
                            pss = ps_s.tile([P, 1024], F32, name="scores")
                            nc.tensor.matmul(
                                pss[:, 0:512],
                                lhsT=kT_sb[0:64, t, ks],
                                rhs=qT_sb[0:64, t, cs],
                                start=True,
                                stop=True,
                            )
                            nc.tensor.matmul(
                                pss[:, 512:1024],
                                lhsT=kT_sb[64:P, t, ks],
                                rhs=qT_sb[64:P, t, cs],
                                start=True,
                                stop=True,
                            )
                            pt = ptp.tile([P, 1024], F16, name="pt")
                            nc.scalar.activation(pt[:], pss[:], EXP, scale=0.125)
                            nc.tensor.matmul(
                                pc_e[:, :],
                                lhsT=v_e[:, sk, t, :],
                                rhs=pt[:, 0:512],
                                start=(sk == 0),
                                stop=(sk == NSK - 1),
                            )
                            nc.tensor.matmul(
                                pc_o[:, :],
                                lhsT=v_o[:, sk, t, :],
                                rhs=pt[:, 512:1024],
                                start=(sk == 0),
                                stop=(sk == NSK - 1),
                            )
                            if sk >= 4:
                                for d in deferred:
                                    if d[0] < idx or (
                                        d[0] == idx and sk >= d[1]
                                    ):
                                        d[2]()
                                        deferred.remove(d)
                                        break
                        # stage psum to SBUF (frees the ctx banks fast),
                        # start the reciprocals, defer the rest two iters
                        u = rcpp.tile([P, 1024], F32, name="u", tag="u")
                        nc.vector.tensor_copy(u[0:65, 0:512], pc_e[0:65, :])
                        nc.vector.tensor_copy(u[:, 512:1024], pc_o[:, :])
                        deferred.append([idx + 1, 12, make_flush(t, cs, u)])
                        if t == 1:
                            for fn in make_outproj(c):
                                deferred.append([idx + 2, 4, fn])
                        idx += 1
                for d in deferred:
                    d[2]()

    n = _split_multi_waits(nc)
    print(f"[kernel] split {n} multi-wait instructions into nops", flush=True)
    return nc


def _in_maps(query, key, value, Wq, bq, Wk, bk, Wv, Wo):
    maps = []
    for core in range(8):
        b = core // 4
        r0 = (core % 4) * M
        r1 = r0 + M
        maps.append(
            {
                "xq": np.ascontiguousarray(query[b].T).astype(np.float16),
                "xk": np.ascontiguousarray(key[b].T).astype(np.float16),
                "xv": np.ascontiguousarray(value[b].T).astype(np.float16),
                "wq": np.ascontiguousarray(Wq[r0:r1].T).astype(np.float16),
                "wk": np.ascontiguousarray(Wk[r0:r1].T).astype(np.float16),
                "wv": np.ascontiguousarray(Wv[r0:r1].T).astype(np.float16),
                "wo": np.ascontiguousarray(Wo[:, r0:r1].T).astype(np.float16),
                "bq": np.ascontiguousarray(bq[r0:r1]),
                "bk": np.ascontiguousarray(bk[r0:r1]),
            }
        )
    return maps


def _gather(results, Wo, bv, bo):
    corr = (bo + Wo @ bv).astype(np.float32)
    full = np.empty((2, S, D), np.float32)
    for b in range(2):
        acc = results[4 * b]["out"].astype(np.float32).copy()
        for i in range(1, 4):
            acc += results[4 * b + i]["out"]
        full[b] = acc + corr[None, :]
    return full


def kernel(query, key, value, Wq, bq, Wk, bk, Wv, bv, Wo, bo, _run_kwargs=None):
    query, key, value, Wq, bq, Wk, bk, Wv, bv, Wo, bo = (
        np.asarray(a, np.float32)
        for a in (query, key, value, Wq, bq, Wk, bk, Wv, bv, Wo, bo)
    )
    nc = _build()
    maps = _in_maps(query, key, value, Wq, bq, Wk, bk, Wv, Wo)
    res = run_bass_kernel_spmd(nc, maps, core_ids=list(range(8)), **(_run_kwargs or {}))
    out = _gather(res.results, Wo, bv, bo)
    if _run_kwargs:
        kernel.last_results = res
    return out
